# revision 6
# baseline (speedup 1.0000x reference)
"""Multi-head attention (B=2, S=4096, D=1024, H=16, HD=64) on 8 trn2 cores.

Sharding: core c -> batch b = c//4, head-group g = c%4 (4 heads per core).
Each core: Q/K/V projections for its heads on its batch, attention, and the
partial output ctx @ Wo[rows of its heads]. Host sums the 4 partials per
batch and adds bo.

v2 (bf16): all matmul operands are bf16 (fp32 streams the PE at 2 cyc/elem;
bf16 at 1), PSUM accumulation stays fp32. Softmax exp is split between the
scalar engine (ACT exp, 5/8 of k-tiles) and the vector engine (Schraudolph
bit-trick exp -> bf16 bit pattern via int16 affine, 3/8 of k-tiles). K's
projection bias is dropped entirely: softmax is invariant to per-query
constants, so only Q needs its bias. PV runs full 128-token contraction per
k-tile (2 PSUM accumulators per head pair), with V carrying a ones column so
the softmax denominator falls out of the same matmul.
"""

import os
from contextlib import ExitStack

import numpy as np

os.environ.setdefault("MYCRO_LOCAL_CACHE", "1")

import concourse.bass as bass
import concourse.tile as tile
from concourse import bacc, mybir
from concourse.bass_utils import run_bass_kernel_spmd
from concourse.masks import make_identity

F32 = mybir.dt.float32
BF16 = mybir.dt.bfloat16
I16 = mybir.dt.int16
AF = mybir.ActivationFunctionType
ALU = mybir.AluOpType

S = 4096          # sequence length
D = 1024          # model dim
HC = 4            # heads per core
HD = 64           # head dim
DC = HC * HD      # 256 per-core projection width
NP = HC // 2      # head pairs per core
KT = S // 128     # 32 k-tiles
QC = S // 512     # 8 q-chunks of 512
T4 = S // 1024    # 4 token chunks of 1024 (phase 1)
SCALE = 1.0 / 8.0
# Schraudolph exp: bf16 bits of exp(s/8) ~= round(s * A + C); bitcast to bf16.
SCHR_A = SCALE * 1.4426950408889634 * 128.0
SCHR_C = 16250.3


def _emit(ctx: ExitStack, tc: tile.TileContext, ins: dict, out: bass.AP):
    nc = tc.nc
    X, Wq, bq, Wk, Wv, bv, Wo = (
        ins["X"], ins["Wq"], ins["bq"], ins["Wk"], ins["Wv"], ins["bv"],
        ins["Wo"],
    )

    const = ctx.enter_context(tc.tile_pool(name="const", bufs=1))
    ident = const.tile([128, 128], F32)
    make_identity(nc, ident[:])

    # Weights: DMA fp32 staging, cast to bf16 once.
    wq_sb = const.tile([128, 8 * DC], BF16, tag="wq")
    wk_sb = const.tile([128, 8 * DC], BF16, tag="wk")
    wv_sb = const.tile([128, 8 * DC], BF16, tag="wv")
    wo_sb = const.tile([128, 2 * D], BF16, tag="wo")
    with tc.tile_pool(name="wstage", bufs=2) as wst:
        for dst, src, nchunks in ((wq_sb, Wq, 8), (wk_sb, Wk, 8),
                                  (wv_sb, Wv, 8), (wo_sb, Wo, 2)):
            stg = wst.tile([128, dst.shape[1]], F32, tag="wstg",
                           name=f"wstg_{src.name}")
            nc.sync.dma_start(stg[:].rearrange("p (c d) -> p c d", c=nchunks),
                              src.rearrange("(c p) d -> p c d", p=128))
            nc.vector.tensor_copy(dst[:], stg[:])
    bq_sb = const.tile([128, 2], F32, tag="bq")
    nc.sync.dma_start(bq_sb[:], bq.rearrange("(c p) -> p c", p=128))
    bv_bc = const.tile([128, DC], F32, tag="bv")
    nc.sync.dma_start(bv_bc[:], bv.unsqueeze(0).to_broadcast([128, DC]))
    ones4 = const.tile([128, HC], F32, tag="ones4")
    nc.vector.memset(ones4[:], 1.0)

    # Activations for phases 1-2 (freed before phase 3).
    acts_ctx = ExitStack()
    acts = acts_ctx.enter_context(tc.tile_pool(name="acts", bufs=1))
    QT = [acts.tile([128, S], BF16, tag=f"qt{p}", name=f"qt{p}") for p in range(NP)]
    KT_ = [acts.tile([128, S], BF16, tag=f"kt{p}", name=f"ktile{p}") for p in range(NP)]
    VPA = acts.tile([128, KT, HC * 65], BF16, tag="vpa", name="vpa")
    VP = [VPA[:, k, :] for k in range(KT)]
    # ctx'' spills to DRAM between phase 2 and phase 3.
    ctx_dram = nc.dram_tensor("ctxs", [HC, 65, S], F32).ap()

    # ---------------- Phase 1: X^T + projections ----------------
    with nc.named_scope("ph1"), \
         tc.tile_pool(name="xrow", bufs=10) as xrow, \
         tc.tile_pool(name="xt", bufs=16) as xtp, \
         tc.tile_pool(name="ps1", bufs=2, space="PSUM") as ps1, \
         tc.tile_pool(name="ps2", bufs=3, space="PSUM") as ps2:
        for t4 in range(T4):
            xts = [xrow.tile([128, D], F32, tag="xr", name=f"xr{t4}_{i}")
                   for i in range(8)]
            for tt in range(8):
                nc.sync.dma_start(xts[tt][:], X[t4 * 1024 + tt * 128:
                                              t4 * 1024 + (tt + 1) * 128, :])
            xt = [xtp.tile([128, 1024], BF16, tag="xt", name=f"xt{t4}_{i}")
                  for i in range(8)]
            for dc in range(8):
                for half in range(2):
                    pt = ps1.tile([128, 512], F32, tag="tp")
                    for q4 in range(4):
                        tt = half * 4 + q4
                        nc.tensor.transpose(
                            pt[:, q4 * 128:(q4 + 1) * 128],
                            xts[tt][:, dc * 128:(dc + 1) * 128], ident[:])
                    dst = xt[dc][:, half * 512:(half + 1) * 512]
                    if (dc + half) % 2 == 0:
                        nc.vector.tensor_copy(dst, pt[:])
                    else:
                        nc.scalar.copy(dst, pt[:])
            for p in range(NP):
                pq = ps2.tile([128, 1024], F32, tag="pq")
                for half in range(2):
                    for dc in range(8):
                        nc.tensor.matmul(
                            pq[:, half * 512:(half + 1) * 512],
                            wq_sb[:, dc * DC + p * 128: dc * DC + (p + 1) * 128],
                            xt[dc][:, half * 512:(half + 1) * 512],
                            start=(dc == 0), stop=(dc == 7))
                nc.vector.tensor_scalar_add(
                    QT[p][:, t4 * 1024:(t4 + 1) * 1024], pq[:], bq_sb[:, p:p + 1])
                pk = ps2.tile([128, 1024], F32, tag="pq")
                for half in range(2):
                    for dc in range(8):
                        nc.tensor.matmul(
                            pk[:, half * 512:(half + 1) * 512],
                            wk_sb[:, dc * DC + p * 128: dc * DC + (p + 1) * 128],
                            xt[dc][:, half * 512:(half + 1) * 512],
                            start=(dc == 0), stop=(dc == 7))
                nc.scalar.copy(KT_[p][:, t4 * 1024:(t4 + 1) * 1024], pk[:])
            for vh in range(2):
                pv = ps2.tile([128, 1024], F32, tag="pq")
                for q4 in range(4):
                    tt = vh * 4 + q4
                    for dc in range(8):
                        nc.tensor.matmul(
                            pv[:, q4 * 256:(q4 + 1) * 256],
                            xt[dc][:, tt * 128:(tt + 1) * 128],
                            wv_sb[:, dc * DC:(dc + 1) * DC],
                            start=(dc == 0), stop=(dc == 7))
                for q4 in range(4):
                    kt = t4 * 8 + vh * 4 + q4
                    vdst = VP[kt].rearrange("p (h w) -> p h w", h=HC)[:, :, 0:64]
                    nc.vector.scalar_tensor_tensor(
                        vdst,
                        pv[:, q4 * 256:(q4 + 1) * 256].rearrange(
                            "p (h w) -> p h w", h=HC), 1.0,
                        bv_bc[:].rearrange("p (h w) -> p h w", h=HC),
                        ALU.bypass, ALU.add)
                    ones = VP[kt].rearrange("p (h w) -> p h w", h=HC)[:, :, 64:65]
                    nc.vector.tensor_copy(ones, ones4[:].unsqueeze(2))

    # ---------------- Phase 2: attention ----------------
    with nc.named_scope("ph2"), \
         tc.tile_pool(name="sps", bufs=3, space="PSUM") as sps, \
         tc.tile_pool(name="pvs", bufs=2, space="PSUM") as pvs, \
         tc.tile_pool(name="et", bufs=8) as etp, \
         tc.tile_pool(name="bsb", bufs=4) as bsb:
        seq = [(p, qc, k) for p in range(NP) for qc in range(QC)
               for k in range(KT)]
        accs = {}
        ets = {}

        def s_step(i):
            p, qc, k = seq[i]
            qs = slice(qc * 512, (qc + 1) * 512)
            ks = slice(k * 128, (k + 1) * 128)
            st = sps.tile([128, 1024], F32, tag="st", name=f"st{p}_{qc}_{k}")
            nc.tensor.matmul(st[:, 0:512], KT_[p][0:64, ks],
                             QT[p][0:64, qs], start=True, stop=True)
            nc.tensor.matmul(st[:, 512:1024], KT_[p][64:128, ks],
                             QT[p][64:128, qs], start=True, stop=True)
            et = etp.tile([128, 1024], BF16, tag="et", name=f"et{p}_{qc}_{k}")
            if k % 2 == 0:
                nc.scalar.activation(et[:], st[:], AF.Exp, bias=0.0,
                                     scale=SCALE)
            else:
                nc.vector.tensor_scalar(et[:].bitcast(I16), st[:],
                                        SCHR_A, SCHR_C, ALU.mult, ALU.add)
            ets[i] = et

        LOOKAHEAD = 3
        for i in range(LOOKAHEAD):
            s_step(i)
        for i, (p, qc, k) in enumerate(seq):
            if i + LOOKAHEAD < len(seq):
                s_step(i + LOOKAHEAD)
            if k == 0:
                accs[(p, qc)] = [
                    pvs.tile([65, 512], F32, tag="acc", name=f"acc{p}_{qc}_{j}")
                    for j in range(2)]
            acc = accs[(p, qc)]
            et = ets.pop(i)
            for j in range(2):
                h = 2 * p + j
                vs = slice(h * 65, (h + 1) * 65)
                es = slice(j * 512, (j + 1) * 512)
                nc.tensor.matmul(
                    acc[j][:], VP[k][:, vs], et[:, es],
                    start=(k == 0), stop=(k == KT - 1),
                    skip_group_check=True)
            if k == KT - 1:
                qs = slice(qc * 512, (qc + 1) * 512)
                for j in range(2):
                    h = 2 * p + j
                    cst = bsb.tile([65, 512], F32, tag="cst",
                                   name=f"cst{p}_{qc}_{j}")
                    if j == 0:
                        nc.vector.tensor_copy(cst[:], acc[j][:])
                    else:
                        nc.scalar.copy(cst[:], acc[j][:])
                    nc.sync.dma_start(ctx_dram[h, :, qs], cst[:])
                del accs[(p, qc)]

    acts_ctx.close()

    # ---------------- Phase 3: normalize + Wo ----------------
    with nc.named_scope("ph3"), \
         tc.tile_pool(name="ps3a", bufs=4, space="PSUM") as ps3a, \
         tc.tile_pool(name="ps3b", bufs=2, space="PSUM") as ps3b, \
         tc.tile_pool(name="po", bufs=2, space="PSUM") as pop, \
         tc.tile_pool(name="ctxn", bufs=3) as ctxnp, \
         tc.tile_pool(name="rcpp", bufs=8) as rcpp, \
         tc.tile_pool(name="ltp", bufs=4) as ltp, \
         tc.tile_pool(name="ctl", bufs=12) as ctl, \
         tc.tile_pool(name="osb", bufs=4) as osbp:
        for t in range(S // 128):
            ts_ = slice(t * 128, (t + 1) * 128)
            ct4 = ctl.tile([65, HC * 128], F32, tag="ct", name=f"ct{t}")
            nc.scalar.dma_start(
                ct4[:].rearrange("p (h w) -> p h w", h=HC),
                ctx_dram[:, :, ts_].transpose([1, 0, 2]))
            cts = [ct4[:, h * 128:(h + 1) * 128] for h in range(HC)]
            lts = []
            for p in range(NP):
                ctxn = ctxnp.tile([128, 128], F32, tag="ctxn",
                                  name=f"ctxn{t}_{p}")
                for j in range(2):
                    h = 2 * p + j
                    tp1 = ps3a.tile([128, 65], F32, tag="tp1",
                                    name=f"tp1_{t}_{h}")
                    nc.tensor.transpose(tp1[:], cts[h][:], ident[0:65, 0:65])
                    rcp = rcpp.tile([128, 1], F32, tag="rcp",
                                    name=f"rcp{t}_{h}")
                    nc.vector.reciprocal(rcp[:], tp1[:, 64:65])
                    nc.vector.tensor_scalar_mul(
                        ctxn[:, j * 64:(j + 1) * 64], tp1[:, 0:64], rcp[:])
                tp2 = ps3b.tile([128, 128], F32, tag="tp2", name=f"tp2_{t}_{p}")
                nc.tensor.transpose(tp2[:], ctxn[:], ident[:])
                lt = ltp.tile([128, 128], BF16, tag="lt", name=f"lt{t}_{p}")
                if p == 0:
                    nc.vector.tensor_copy(lt[:], tp2[:])
                else:
                    nc.scalar.copy(lt[:], tp2[:])
                lts.append(lt)
            ot = osbp.tile([128, D], F32, tag="ot", name=f"ot{t}")
            for n2 in range(2):
                po = pop.tile([128, 512], F32, tag="po", name=f"po{t}_{n2}")
                for p in range(NP):
                    nc.tensor.matmul(
                        po[:], lts[p][:],
                        wo_sb[:, p * D + n2 * 512: p * D + (n2 + 1) * 512],
                        start=(p == 0), stop=(p == NP - 1))
                if n2 == 0:
                    nc.scalar.copy(ot[:, 0:512], po[:])
                else:
                    nc.vector.tensor_copy(ot[:, 512:1024], po[:])
            nc.sync.dma_start(out[ts_, :], ot[:])

_CACHE = {}


def _build():
    if "nc" in _CACHE:
        return _CACHE["nc"]
    nc = bacc.Bacc("TRN2", target_bir_lowering=False, debug=False)
    ins = {
        "X": nc.dram_tensor("X", [S, D], F32, kind="ExternalInput").ap(),
        "Wq": nc.dram_tensor("Wq", [D, DC], F32, kind="ExternalInput").ap(),
        "bq": nc.dram_tensor("bq", [DC], F32, kind="ExternalInput").ap(),
        "Wk": nc.dram_tensor("Wk", [D, DC], F32, kind="ExternalInput").ap(),
        "Wv": nc.dram_tensor("Wv", [D, DC], F32, kind="ExternalInput").ap(),
        "bv": nc.dram_tensor("bv", [DC], F32, kind="ExternalInput").ap(),
        "Wo": nc.dram_tensor("Wo", [DC, D], F32, kind="ExternalInput").ap(),
    }
    outp = nc.dram_tensor("out", [S, D], F32, kind="ExternalOutput").ap()
    with tile.TileContext(nc) as tcx:
        with ExitStack() as ctx:
            _emit(ctx, tcx, ins, outp)
    nc.compile()
    _CACHE["nc"] = nc
    return nc


def core_inputs(X, Wq, bq, Wk, bk, Wv, bv, Wo, core):
    b, g = divmod(core, 4)
    cs = slice(g * DC, (g + 1) * DC)
    return {
        "X": np.ascontiguousarray(X[b]),
        "Wq": np.ascontiguousarray(Wq[:, cs]), "bq": np.ascontiguousarray(bq[cs]),
        "Wk": np.ascontiguousarray(Wk[:, cs]),
        "Wv": np.ascontiguousarray(Wv[:, cs]), "bv": np.ascontiguousarray(bv[cs]),
        "Wo": np.ascontiguousarray(Wo[cs, :]),
    }


def kernel(X, Wq, bq, Wk, bk, Wv, bv, Wo, bo, _trace=False):
    nc = _build()
    in_maps = [core_inputs(X, Wq, bq, Wk, bk, Wv, bv, Wo, c) for c in range(8)]
    res = run_bass_kernel_spmd(nc, in_maps, list(range(8)), trace=_trace)
    parts = [res.results[c]["out"] for c in range(8)]
    full = np.stack([
        parts[0] + parts[1] + parts[2] + parts[3] + bo,
        parts[4] + parts[5] + parts[6] + parts[7] + bo,
    ]).astype(np.float32)
    if _trace:
        return full, res
    return full


# revision 8
# speedup vs baseline: 1.1926x; 1.1926x over previous
"""Multi-head attention (B=2, S=4096, D=1024, H=16, HD=64) on 8 trn2 cores.

Sharding: core c -> batch b = c//4, head-group g = c%4 (4 heads per core).
Each core: Q/K/V projections for its heads on its batch, attention, and the
partial output ctx @ Wo[rows of its heads]. Host sums the 4 partials per
batch and adds bo.

v2 (bf16): all matmul operands are bf16 (fp32 streams the PE at 2 cyc/elem;
bf16 at 1), PSUM accumulation stays fp32. Softmax exp is split between the
scalar engine (ACT exp, 5/8 of k-tiles) and the vector engine (Schraudolph
bit-trick exp -> bf16 bit pattern via int16 affine, 3/8 of k-tiles). K's
projection bias is dropped entirely: softmax is invariant to per-query
constants, so only Q needs its bias. PV runs full 128-token contraction per
k-tile (2 PSUM accumulators per head pair), with V carrying a ones column so
the softmax denominator falls out of the same matmul.
"""

import os
from contextlib import ExitStack

import numpy as np

os.environ.setdefault("MYCRO_LOCAL_CACHE", "1")

import concourse.bass as bass
import concourse.tile as tile
from concourse import bacc, mybir
from concourse.bass_utils import run_bass_kernel_spmd
from concourse.masks import make_identity

F32 = mybir.dt.float32
BF16 = mybir.dt.bfloat16
I16 = mybir.dt.int16
AF = mybir.ActivationFunctionType
ALU = mybir.AluOpType

S = 4096          # sequence length
D = 1024          # model dim
HC = 4            # heads per core
HD = 64           # head dim
DC = HC * HD      # 256 per-core projection width
NP = HC // 2      # head pairs per core
KT = S // 128     # 32 k-tiles
QC = S // 512     # 8 q-chunks of 512
T4 = S // 1024    # 4 token chunks of 1024 (phase 1)
SCALE = 1.0 / 8.0
# Schraudolph exp: bf16 bits of exp(s/8) ~= round(s * A + C); bitcast to bf16.
SCHR_A = SCALE * 1.4426950408889634 * 128.0
SCHR_C = 16250.3


def _emit(ctx: ExitStack, tc: tile.TileContext, ins: dict, out: bass.AP):
    nc = tc.nc
    X, Wq, bq, Wk, Wv, bv, Wo = (
        ins["X"], ins["Wq"], ins["bq"], ins["Wk"], ins["Wv"], ins["bv"],
        ins["Wo"],
    )

    const = ctx.enter_context(tc.tile_pool(name="const", bufs=1))
    ident = const.tile([128, 128], F32)
    make_identity(nc, ident[:])

    # Weights: DMA fp32 staging, cast to bf16 once.
    wq_sb = const.tile([128, 8 * DC], BF16, tag="wq")
    wk_sb = const.tile([128, 8 * DC], BF16, tag="wk")
    wv_sb = const.tile([128, 8 * DC], BF16, tag="wv")
    wo_sb = const.tile([128, 2 * D], BF16, tag="wo")
    with tc.tile_pool(name="wstage", bufs=2) as wst:
        for dst, src, nchunks in ((wq_sb, Wq, 8), (wk_sb, Wk, 8),
                                  (wv_sb, Wv, 8), (wo_sb, Wo, 2)):
            stg = wst.tile([128, dst.shape[1]], F32, tag="wstg",
                           name=f"wstg_{src.name}")
            nc.sync.dma_start(stg[:].rearrange("p (c d) -> p c d", c=nchunks),
                              src.rearrange("(c p) d -> p c d", p=128))
            nc.vector.tensor_copy(dst[:], stg[:])
    bq_sb = const.tile([128, 2], F32, tag="bq")
    nc.sync.dma_start(bq_sb[:], bq.rearrange("(c p) -> p c", p=128))
    bv_bc = const.tile([128, DC], F32, tag="bv")
    nc.sync.dma_start(bv_bc[:], bv.unsqueeze(0).to_broadcast([128, DC]))
    ones4 = const.tile([128, HC], F32, tag="ones4")
    nc.vector.memset(ones4[:], 1.0)

    # Activations for phases 1-2 (freed before phase 3).
    acts_ctx = ExitStack()
    acts = acts_ctx.enter_context(tc.tile_pool(name="acts", bufs=1))
    QT = [acts.tile([128, S], BF16, tag=f"qt{p}", name=f"qt{p}") for p in range(NP)]
    KT_ = [acts.tile([128, S], BF16, tag=f"kt{p}", name=f"ktile{p}") for p in range(NP)]
    VPA = acts.tile([128, KT, HC * 65], BF16, tag="vpa", name="vpa")
    VP = [VPA[:, k, :] for k in range(KT)]
    # ctx'' spills to DRAM between phase 2 and phase 3.
    ctx_dram = nc.dram_tensor("ctxs", [HC, 65, S], F32).ap()

    # ---------------- Phase 1: X^T + projections ----------------
    with nc.named_scope("ph1"), \
         tc.tile_pool(name="xrow", bufs=10) as xrow, \
         tc.tile_pool(name="xt", bufs=16) as xtp, \
         tc.tile_pool(name="ps1", bufs=2, space="PSUM") as ps1, \
         tc.tile_pool(name="ps2", bufs=3, space="PSUM") as ps2:
        for t4 in range(T4):
            xts = [xrow.tile([128, D], F32, tag="xr", name=f"xr{t4}_{i}")
                   for i in range(8)]
            for tt in range(8):
                nc.sync.dma_start(xts[tt][:], X[t4 * 1024 + tt * 128:
                                              t4 * 1024 + (tt + 1) * 128, :])
            xt = [xtp.tile([128, 1024], BF16, tag="xt", name=f"xt{t4}_{i}")
                  for i in range(8)]
            for dc in range(8):
                for half in range(2):
                    pt = ps1.tile([128, 512], F32, tag="tp")
                    for q4 in range(4):
                        tt = half * 4 + q4
                        nc.tensor.transpose(
                            pt[:, q4 * 128:(q4 + 1) * 128],
                            xts[tt][:, dc * 128:(dc + 1) * 128], ident[:])
                    dst = xt[dc][:, half * 512:(half + 1) * 512]
                    if (dc + half) % 2 == 0:
                        nc.vector.tensor_copy(dst, pt[:])
                    else:
                        nc.scalar.copy(dst, pt[:])
            for p in range(NP):
                pq = ps2.tile([128, 1024], F32, tag="pq")
                for half in range(2):
                    for dc in range(8):
                        nc.tensor.matmul(
                            pq[:, half * 512:(half + 1) * 512],
                            wq_sb[:, dc * DC + p * 128: dc * DC + (p + 1) * 128],
                            xt[dc][:, half * 512:(half + 1) * 512],
                            start=(dc == 0), stop=(dc == 7))
                nc.vector.tensor_scalar_add(
                    QT[p][:, t4 * 1024:(t4 + 1) * 1024], pq[:], bq_sb[:, p:p + 1])
                pk = ps2.tile([128, 1024], F32, tag="pq")
                for half in range(2):
                    for dc in range(8):
                        nc.tensor.matmul(
                            pk[:, half * 512:(half + 1) * 512],
                            wk_sb[:, dc * DC + p * 128: dc * DC + (p + 1) * 128],
                            xt[dc][:, half * 512:(half + 1) * 512],
                            start=(dc == 0), stop=(dc == 7))
                nc.scalar.copy(KT_[p][:, t4 * 1024:(t4 + 1) * 1024], pk[:])
            for vh in range(2):
                pv = ps2.tile([128, 1024], F32, tag="pq")
                for q4 in range(4):
                    tt = vh * 4 + q4
                    for dc in range(8):
                        nc.tensor.matmul(
                            pv[:, q4 * 256:(q4 + 1) * 256],
                            xt[dc][:, tt * 128:(tt + 1) * 128],
                            wv_sb[:, dc * DC:(dc + 1) * DC],
                            start=(dc == 0), stop=(dc == 7))
                for q4 in range(4):
                    kt = t4 * 8 + vh * 4 + q4
                    vdst = VP[kt].rearrange("p (h w) -> p h w", h=HC)[:, :, 0:64]
                    nc.vector.scalar_tensor_tensor(
                        vdst,
                        pv[:, q4 * 256:(q4 + 1) * 256].rearrange(
                            "p (h w) -> p h w", h=HC), 1.0,
                        bv_bc[:].rearrange("p (h w) -> p h w", h=HC),
                        ALU.bypass, ALU.add)
                    ones = VP[kt].rearrange("p (h w) -> p h w", h=HC)[:, :, 64:65]
                    nc.vector.tensor_copy(ones, ones4[:].unsqueeze(2))

    # ---------------- Phase 2: attention ----------------
    with nc.named_scope("ph2"), \
         tc.tile_pool(name="sps", bufs=3, space="PSUM") as sps, \
         tc.tile_pool(name="pvs", bufs=2, space="PSUM") as pvs, \
         tc.tile_pool(name="et", bufs=10) as etp, \
         tc.tile_pool(name="bsb", bufs=4) as bsb:
        seq = [(p, qc, k) for p in range(NP) for qc in range(QC)
               for k in range(KT)]
        accs = {}
        ets = {}

        def s_step(i):
            p, qc, k = seq[i]
            qs = slice(qc * 512, (qc + 1) * 512)
            ks = slice(k * 128, (k + 1) * 128)
            st = sps.tile([128, 1024], F32, tag="st", name=f"st{p}_{qc}_{k}")
            nc.tensor.matmul(st[:, 0:512], KT_[p][0:64, ks],
                             QT[p][0:64, qs], start=True, stop=True)
            nc.tensor.matmul(st[:, 512:1024], KT_[p][64:128, ks],
                             QT[p][64:128, qs], start=True, stop=True)
            et = etp.tile([128, 1024], BF16, tag="et", name=f"et{p}_{qc}_{k}")
            for hf in range(2):
                hs = slice(hf * 512, (hf + 1) * 512)
                if k % 2 == 0:
                    nc.scalar.activation(et[:, hs], st[:, hs], AF.Exp,
                                         bias=0.0, scale=SCALE)
                else:
                    nc.vector.tensor_scalar(et[:, hs].bitcast(I16), st[:, hs],
                                            SCHR_A, SCHR_C, ALU.mult, ALU.add)
            ets[i] = et

        LOOKAHEAD = 4
        for i in range(LOOKAHEAD):
            s_step(i)
        for i, (p, qc, k) in enumerate(seq):
            if i + LOOKAHEAD < len(seq):
                s_step(i + LOOKAHEAD)
            if k == 0:
                accs[(p, qc)] = [
                    pvs.tile([65, 512], F32, tag="acc", name=f"acc{p}_{qc}_{j}")
                    for j in range(2)]
            acc = accs[(p, qc)]
            et = ets.pop(i)
            for j in range(2):
                h = 2 * p + j
                vs = slice(h * 65, (h + 1) * 65)
                es = slice(j * 512, (j + 1) * 512)
                nc.tensor.matmul(
                    acc[j][:], VP[k][:, vs], et[:, es],
                    start=(k == 0), stop=(k == KT - 1),
                    skip_group_check=True)
            if k == KT - 1:
                qs = slice(qc * 512, (qc + 1) * 512)
                for j in range(2):
                    h = 2 * p + j
                    cst = bsb.tile([65, 512], F32, tag="cst",
                                   name=f"cst{p}_{qc}_{j}")
                    if j == 0:
                        nc.vector.tensor_copy(cst[:], acc[j][:])
                    else:
                        nc.scalar.copy(cst[:], acc[j][:])
                    nc.sync.dma_start(ctx_dram[h, :, qs], cst[:])
                del accs[(p, qc)]

    acts_ctx.close()

    # ---------------- Phase 3: normalize + Wo ----------------
    with nc.named_scope("ph3"), \
         tc.tile_pool(name="ps3a", bufs=4, space="PSUM") as ps3a, \
         tc.tile_pool(name="ps3b", bufs=2, space="PSUM") as ps3b, \
         tc.tile_pool(name="po", bufs=2, space="PSUM") as pop, \
         tc.tile_pool(name="ctxn", bufs=3) as ctxnp, \
         tc.tile_pool(name="rcpp", bufs=8) as rcpp, \
         tc.tile_pool(name="ltp", bufs=4) as ltp, \
         tc.tile_pool(name="ctl", bufs=12) as ctl, \
         tc.tile_pool(name="osb", bufs=4) as osbp:
        for t in range(S // 128):
            ts_ = slice(t * 128, (t + 1) * 128)
            ct4 = ctl.tile([65, HC * 128], F32, tag="ct", name=f"ct{t}")
            nc.scalar.dma_start(
                ct4[:].rearrange("p (h w) -> p h w", h=HC),
                ctx_dram[:, :, ts_].transpose([1, 0, 2]))
            cts = [ct4[:, h * 128:(h + 1) * 128] for h in range(HC)]
            lts = []
            for p in range(NP):
                ctxn = ctxnp.tile([128, 128], F32, tag="ctxn",
                                  name=f"ctxn{t}_{p}")
                for j in range(2):
                    h = 2 * p + j
                    tp1 = ps3a.tile([128, 65], F32, tag="tp1",
                                    name=f"tp1_{t}_{h}")
                    nc.tensor.transpose(tp1[:], cts[h][:], ident[0:65, 0:65])
                    rcp = rcpp.tile([128, 1], F32, tag="rcp",
                                    name=f"rcp{t}_{h}")
                    nc.vector.reciprocal(rcp[:], tp1[:, 64:65])
                    nc.vector.tensor_scalar_mul(
                        ctxn[:, j * 64:(j + 1) * 64], tp1[:, 0:64], rcp[:])
                tp2 = ps3b.tile([128, 128], F32, tag="tp2", name=f"tp2_{t}_{p}")
                nc.tensor.transpose(tp2[:], ctxn[:], ident[:])
                lt = ltp.tile([128, 128], BF16, tag="lt", name=f"lt{t}_{p}")
                if p == 0:
                    nc.vector.tensor_copy(lt[:], tp2[:])
                else:
                    nc.scalar.copy(lt[:], tp2[:])
                lts.append(lt)
            ot = osbp.tile([128, D], F32, tag="ot", name=f"ot{t}")
            for n2 in range(2):
                po = pop.tile([128, 512], F32, tag="po", name=f"po{t}_{n2}")
                for p in range(NP):
                    nc.tensor.matmul(
                        po[:], lts[p][:],
                        wo_sb[:, p * D + n2 * 512: p * D + (n2 + 1) * 512],
                        start=(p == 0), stop=(p == NP - 1))
                if n2 == 0:
                    nc.scalar.copy(ot[:, 0:512], po[:])
                else:
                    nc.vector.tensor_copy(ot[:, 512:1024], po[:])
            nc.sync.dma_start(out[ts_, :], ot[:])

_CACHE = {}


def _build():
    if "nc" in _CACHE:
        return _CACHE["nc"]
    nc = bacc.Bacc("TRN2", target_bir_lowering=False, debug=False)
    ins = {
        "X": nc.dram_tensor("X", [S, D], F32, kind="ExternalInput").ap(),
        "Wq": nc.dram_tensor("Wq", [D, DC], F32, kind="ExternalInput").ap(),
        "bq": nc.dram_tensor("bq", [DC], F32, kind="ExternalInput").ap(),
        "Wk": nc.dram_tensor("Wk", [D, DC], F32, kind="ExternalInput").ap(),
        "Wv": nc.dram_tensor("Wv", [D, DC], F32, kind="ExternalInput").ap(),
        "bv": nc.dram_tensor("bv", [DC], F32, kind="ExternalInput").ap(),
        "Wo": nc.dram_tensor("Wo", [DC, D], F32, kind="ExternalInput").ap(),
    }
    outp = nc.dram_tensor("out", [S, D], F32, kind="ExternalOutput").ap()
    with tile.TileContext(nc) as tcx:
        with ExitStack() as ctx:
            _emit(ctx, tcx, ins, outp)
    nc.compile()
    _CACHE["nc"] = nc
    return nc


def core_inputs(X, Wq, bq, Wk, bk, Wv, bv, Wo, core):
    b, g = divmod(core, 4)
    cs = slice(g * DC, (g + 1) * DC)
    return {
        "X": np.ascontiguousarray(X[b]),
        "Wq": np.ascontiguousarray(Wq[:, cs]), "bq": np.ascontiguousarray(bq[cs]),
        "Wk": np.ascontiguousarray(Wk[:, cs]),
        "Wv": np.ascontiguousarray(Wv[:, cs]), "bv": np.ascontiguousarray(bv[cs]),
        "Wo": np.ascontiguousarray(Wo[cs, :]),
    }


def kernel(X, Wq, bq, Wk, bk, Wv, bv, Wo, bo, _trace=False):
    nc = _build()
    in_maps = [core_inputs(X, Wq, bq, Wk, bk, Wv, bv, Wo, c) for c in range(8)]
    res = run_bass_kernel_spmd(nc, in_maps, list(range(8)), trace=_trace)
    parts = [res.results[c]["out"] for c in range(8)]
    full = np.stack([
        parts[0] + parts[1] + parts[2] + parts[3] + bo,
        parts[4] + parts[5] + parts[6] + parts[7] + bo,
    ]).astype(np.float32)
    if _trace:
        return full, res
    return full
